# revision 6
# baseline (speedup 1.0000x reference)
"""Trainium2 Bass kernel for a single-step attention GRU decoder cell.

Math:
  emb_vec = relu(emb[idx]);  q = hidden[-1]
  feat = tanh(q@Wq + enc@Wk); scores = feat@wv; attn = softmax(scores, axis=L)
  context = attn^T @ enc
  x = [context; emb_vec];  h0n = GRU0(x, hidden[0]);  h1n = GRU1(h0n, hidden[1])
  logp = log_softmax(h1n @ Wd + bd)

Sharding over 8 NeuronCores:
  - attention: enc L-sharded (512 rows/core); AllReduce A combines the partial
    unnormalized context [1024], the partial softmax denominator Z, and the
    (input-column-sharded) gh1 partial for GRU layer 1.
  - GRU layer0: output rows sharded (128 rows of h0n per core, no collective).
  - GRU layer1: input(h0n)-column sharded; AllReduce B combines the gi1
    partial -> every core computes the full h1n.
  - dense: vocab sharded (6400 padded cols/core); AllReduce C combines the
    softmax denominator (a single scalar).
Weights are cast to bf16 on the host (matmul inputs); accumulation, softmax
and gate math stay f32. DMAs are ordered by critical path: packed small
inputs, enc/Wk chunks (feat), Wq, encL, GRU weights, then Wd chunks consumed
k-outer by the dense GEMV.
"""

import numpy as np
import ml_dtypes

NCORES = 8
H = 1024
HT = H // 128          # 8 h-tiles
L = 4096
LC = L // NCORES       # 512 rows per core
LT = LC // 128         # 4 l-tiles per core
V = 50257
VC = 6400              # padded vocab cols per core
VT = VC // 128         # 50 v-tiles per core
VPAD = VC * NCORES     # 51200

# packed small-input column layout (f32 [128, NSMALL])
C_QV, C_H0V, C_EMBV = 0, 8, 16
C_H0S, C_H1S = 24, 25
C_WV = 26
C_BIH0, C_BHH0 = 34, 37
C_BIH1, C_BHH1 = 40, 64
C_BD = 88
NSMALL = 138

BF16 = ml_dtypes.bfloat16

_CACHE = {}


def _to_sb(a, kt, f):
    """[kt*128, f] -> SBUF layout [128, kt*f] (k-tile-major along free dim)."""
    return np.ascontiguousarray(
        a.reshape(kt, 128, f).transpose(1, 0, 2).reshape(128, kt * f)
    )


def build_nc(reps=1, parts="full"):
    import concourse.bacc as bacc
    import concourse.mybir as mybir
    import concourse.tile as tile

    f32 = mybir.dt.float32
    bf16 = mybir.dt.bfloat16
    AF = mybir.ActivationFunctionType

    nc = bacc.Bacc("TRN2", target_bir_lowering=False, debug=False,
                   num_devices=NCORES)

    def din(name, shape, dt=bf16):
        return nc.dram_tensor(name, shape, dt, kind="ExternalInput")

    small_d = din("small", [128, NSMALL], f32)
    encT_d = din("encT", [128, HT * LC])          # enc_c.T   (feat rhs)
    encL_d = din("encL", [128, LT * H])           # enc_c     (context lhsT)
    wk_d = din("wk", [128, HT * H])
    wq_d = din("wq", [128, HT * H])
    wih0_d = din("wih0", [128, 16 * 384])
    whh0_d = din("whh0", [128, 8 * 384])
    wih1_d = din("wih1", [128, 3072])
    whh1_d = din("whh1", [128, 3072])
    wd_d = din("wd", [128, HT * VC])

    out_logp_d = nc.dram_tensor("out_logp", [128, VT], f32, kind="ExternalOutput")
    out_h0n_d = nc.dram_tensor("out_h0n", [128, 1], f32, kind="ExternalOutput")
    out_h1n_d = nc.dram_tensor("out_h1n", [128, HT], f32, kind="ExternalOutput")

    RG = [list(range(NCORES))]

    with tile.TileContext(nc) as tc:
        with (
            tc.tile_pool(name="inp", bufs=1) as inp,
            tc.tile_pool(name="wrk", bufs=1) as wrk,
            tc.tile_pool(name="ps_feat", bufs=2, space="PSUM") as ps_feat,
            tc.tile_pool(name="ps_col", bufs=2, space="PSUM") as ps_col,
            tc.tile_pool(name="ps_tiny", bufs=2, space="PSUM") as ps_tiny,
            tc.tile_pool(name="dram", bufs=1, space="DRAM") as dram,
        ):
            for _rep in range(reps):
                def sb(name, shape, dt):
                    return inp.tile(shape, dt, tag=name, name=name)

                # ---- DMAs, critical-path order ----
                small = sb("small", [128, NSMALL], f32)
                nc.sync.dma_start(small[:], small_d[:, :])
                encT = sb("encT", [128, HT * LC], bf16)
                wk = sb("wk", [128, HT * H], bf16)
                for k in range(HT):
                    nc.sync.dma_start(encT[:, k * LC:(k + 1) * LC],
                                      encT_d[:, k * LC:(k + 1) * LC])
                    nc.sync.dma_start(wk[:, k * H:(k + 1) * H],
                                      wk_d[:, k * H:(k + 1) * H])
                wq = sb("wq", [128, HT * H], bf16)
                for k in range(0, HT, 4):
                    nc.sync.dma_start(wq[:, k * H:(k + 4) * H],
                                      wq_d[:, k * H:(k + 4) * H])
                encL = sb("encL", [128, LT * H], bf16)
                nc.sync.dma_start(encL[:], encL_d[:, :])
                whh1 = sb("whh1", [128, 3072], bf16)
                nc.sync.dma_start(whh1[:], whh1_d[:, :])
                wih0 = sb("wih0", [128, 16 * 384], bf16)
                nc.sync.dma_start(wih0[:], wih0_d[:, :])
                whh0 = sb("whh0", [128, 8 * 384], bf16)
                nc.sync.dma_start(whh0[:], whh0_d[:, :])
                wih1 = sb("wih1", [128, 3072], bf16)
                nc.sync.dma_start(wih1[:], wih1_d[:, :])
                # Wd in 5 vocab-groups x 8 k-chunks so the dense GEMV can
                # finish each vocab-group as soon as its slice lands.
                wd = sb("wd", [128, HT * VC], bf16)
                MG = VC // 5                     # 1280 cols per vocab-group
                for mg in range(5):
                    for k in range(HT):
                        nc.sync.dma_start(
                            wd[:, k * VC + mg * MG:k * VC + (mg + 1) * MG],
                            wd_d[:, k * VC + mg * MG:k * VC + (mg + 1) * MG])
                if parts == "dma":
                    continue

                # ---- constants ----
                ones_row = wrk.tile([1, 128], bf16, tag="ones_row", name="ones_row")
                nc.vector.memset(ones_row[:], 1.0)
                ones_col = wrk.tile([128, 1], bf16, tag="ones_col", name="ones_col")
                nc.vector.memset(ones_col[:], 1.0)
                negones_f = wrk.tile([1, 128], f32, tag="negones_f", name="negones_f")
                nc.vector.memset(negones_f[:], -1.0)

                # ---- casts ----
                q_bf = wrk.tile([128, HT], bf16, tag="q_bf", name="q_bf")
                nc.vector.tensor_copy(q_bf[:], small[:, C_QV:C_QV + 8])
                h0_bf = wrk.tile([128, HT], bf16, tag="h0_bf", name="h0_bf")
                nc.vector.tensor_copy(h0_bf[:], small[:, C_H0V:C_H0V + 8])
                emb_bf = wrk.tile([128, HT], bf16, tag="emb_bf", name="emb_bf")
                nc.scalar.activation(emb_bf[:], small[:, C_EMBV:C_EMBV + 8], AF.Relu)
                h1s_bf = wrk.tile([128, 1], bf16, tag="h1s_bf", name="h1s_bf")
                nc.vector.tensor_copy(h1s_bf[:], small[:, C_H1S:C_H1S + 1])
                wv_bf = wrk.tile([128, HT], bf16, tag="wv_bf", name="wv_bf")
                nc.vector.tensor_copy(wv_bf[:], small[:, C_WV:C_WV + 8])

                # ---- qWq:  qwq[h] = sum_k q[k] Wq[k, h]  -> [128, 8] ----
                p_qwq = ps_col.tile([128, HT], f32, tag="colA", name="p_qwq")
                for m in range(HT):
                    for k in range(HT):
                        nc.tensor.matmul(
                            p_qwq[:, m:m + 1],
                            wq[:, k * H + m * 128:k * H + (m + 1) * 128],
                            q_bf[:, k:k + 1],
                            start=(k == 0), stop=(k == HT - 1))
                qwq = wrk.tile([128, HT], f32, tag="qwq", name="qwq")
                nc.vector.tensor_copy(qwq[:], p_qwq[:])

                # ---- feat = tanh(qWq + enc@Wk), layout [h-part, l-free] ----
                feat = wrk.tile([128, HT * LC], bf16, tag="feat", name="feat")
                for m in range(HT):
                    p_feat = ps_feat.tile([128, LC], f32, tag="feat",
                                          name=f"p_feat{m}")
                    for k in range(HT):
                        nc.tensor.matmul(
                            p_feat[:],
                            wk[:, k * H + m * 128:k * H + (m + 1) * 128],
                            encT[:, k * LC:(k + 1) * LC],
                            start=(k == 0), stop=(k == HT - 1))
                    nc.scalar.activation(feat[:, m * LC:(m + 1) * LC], p_feat[:],
                                         AF.Tanh, bias=qwq[:, m:m + 1])

                # ---- scores (l-partition): s = feat^T @ wv -> [128, 4] ----
                p_sc = ps_col.tile([128, LT], f32, tag="colA", name="p_sc")
                for lt in range(LT):
                    for k in range(HT):
                        nc.tensor.matmul(
                            p_sc[:, lt:lt + 1],
                            feat[:, k * LC + lt * 128:k * LC + (lt + 1) * 128],
                            wv_bf[:, k:k + 1],
                            start=(k == 0), stop=(k == HT - 1))
                esc = wrk.tile([128, LT], bf16, tag="esc", name="esc")
                nc.scalar.activation(esc[:], p_sc[:], AF.Exp)

                # ---- partial context + partial Z -> [128, 9] (col 8 = Z) ----
                p_ctx = ps_col.tile([128, HT + 1], f32, tag="colA", name="p_ctx")
                for m in range(HT):
                    for lt in range(LT):
                        nc.tensor.matmul(
                            p_ctx[:, m:m + 1],
                            encL[:, lt * H + m * 128:lt * H + (m + 1) * 128],
                            esc[:, lt:lt + 1],
                            start=(lt == 0), stop=(lt == LT - 1))
                for lt in range(LT):
                    nc.tensor.matmul(
                        p_ctx[0:1, HT:HT + 1],
                        ones_col[:],
                        esc[:, lt:lt + 1],
                        start=(lt == 0), stop=(lt == LT - 1))
                ctxz = wrk.tile([128, HT + 1], f32, tag="ctxz", name="ctxz")
                nc.vector.memset(ctxz[:], 0.0)
                nc.vector.tensor_copy(ctxz[:, 0:HT], p_ctx[:, 0:HT])
                nc.vector.tensor_copy(ctxz[0:1, HT:HT + 1], p_ctx[0:1, HT:HT + 1])

                # ---- gh1 partial (depends only on whh1 + h1 slice) ----
                p_p2 = ps_col.tile([128, 24], f32, tag="colA", name="p_p2")
                for m in range(24):
                    nc.tensor.matmul(p_p2[:, m:m + 1],
                                     whh1[:, m * 128:(m + 1) * 128],
                                     h1s_bf[:], start=True, stop=True)
                p2s = wrk.tile([128, 24], f32, tag="p2s", name="p2s")
                nc.vector.tensor_copy(p2s[:], p_p2[:])

                # ---- AllReduce A: [ctx(8) | Z(1) | gh1_partial(24)] ----
                ccA_in = dram.tile([128, 33], f32, tag="ccA_in", name="ccA_in")
                ccA_out = dram.tile([128, 33], f32, tag="ccA_out", name="ccA_out")
                nc.sync.dma_start(ccA_in[:, 0:9], ctxz[:])
                nc.sync.dma_start(ccA_in[:, 9:33], p2s[:])
                if parts == "nocc":
                    nc.sync.dma_start(ccA_out[:], ccA_in[:])
                else:
                    nc.gpsimd.collective_compute(
                        "AllReduce", mybir.AluOpType.add, replica_groups=RG,
                        ins=[ccA_in[:].opt()], outs=[ccA_out[:].opt()])
                ctxz_r = wrk.tile([128, 9], f32, tag="ctxz_r", name="ctxz_r")
                nc.sync.dma_start(ctxz_r[:], ccA_out[:, 0:9])
                gh1r = wrk.tile([128, 24], f32, tag="gh1r", name="gh1r")
                nc.sync.dma_start(gh1r[:], ccA_out[:, 9:33])

                # ---- context = ctx_sum / Z ----
                zrec = wrk.tile([1, 1], f32, tag="zrec", name="zrec")
                nc.vector.reciprocal(zrec[:], ctxz_r[0:1, HT:HT + 1])
                zrec_bf = wrk.tile([1, 1], bf16, tag="zrec_bf", name="zrec_bf")
                nc.vector.tensor_copy(zrec_bf[:], zrec[:])
                p_zb = ps_tiny.tile([128, 1], f32, tag="tiny", name="p_zb")
                nc.tensor.matmul(p_zb[:], ones_row[:], zrec_bf[:],
                                 start=True, stop=True)
                zinv = wrk.tile([128, 1], f32, tag="zinv", name="zinv")
                nc.vector.tensor_copy(zinv[:], p_zb[:])
                ctx_bf = wrk.tile([128, HT], bf16, tag="ctx_bf", name="ctx_bf")
                nc.vector.tensor_scalar_mul(ctx_bf[:], ctxz_r[:, 0:HT], zinv[:])

                if parts == "attn":
                    continue

                # ---- GRU layer 0 (rows sharded) ----
                p_gi0 = ps_col.tile([128, 3], f32, tag="colA", name="p_gi0")
                for g in range(3):
                    for k in range(16):
                        rhs = (ctx_bf[:, k:k + 1] if k < 8
                               else emb_bf[:, k - 8:k - 7])
                        nc.tensor.matmul(
                            p_gi0[:, g:g + 1],
                            wih0[:, k * 384 + g * 128:k * 384 + (g + 1) * 128],
                            rhs, start=(k == 0), stop=(k == 15))
                p_gh0 = ps_col.tile([128, 3], f32, tag="colA", name="p_gh0")
                for g in range(3):
                    for k in range(8):
                        nc.tensor.matmul(
                            p_gh0[:, g:g + 1],
                            whh0[:, k * 384 + g * 128:k * 384 + (g + 1) * 128],
                            h0_bf[:, k:k + 1], start=(k == 0), stop=(k == 7))
                gi0 = wrk.tile([128, 3], f32, tag="gi0", name="gi0")
                nc.vector.tensor_add(gi0[:], p_gi0[:], small[:, C_BIH0:C_BIH0 + 3])
                gh0 = wrk.tile([128, 3], f32, tag="gh0", name="gh0")
                nc.vector.tensor_add(gh0[:], p_gh0[:], small[:, C_BHH0:C_BHH0 + 3])

                t_rz0 = wrk.tile([128, 2], f32, tag="t_rz0", name="t_rz0")
                nc.vector.tensor_add(t_rz0[:], gi0[:, 0:2], gh0[:, 0:2])
                rz0 = wrk.tile([128, 2], f32, tag="rz0", name="rz0")
                nc.scalar.activation(rz0[:], t_rz0[:], AF.Sigmoid)
                t_n0 = wrk.tile([128, 1], f32, tag="t_n0", name="t_n0")
                nc.vector.tensor_mul(t_n0[:], rz0[:, 0:1], gh0[:, 2:3])
                nc.vector.tensor_add(t_n0[:], t_n0[:], gi0[:, 2:3])
                n0 = wrk.tile([128, 1], f32, tag="n0", name="n0")
                nc.scalar.activation(n0[:], t_n0[:], AF.Tanh)
                # h0n = n0 + z*(h_prev - n0)
                t_d0 = wrk.tile([128, 1], f32, tag="t_d0", name="t_d0")
                nc.vector.tensor_sub(t_d0[:], small[:, C_H0S:C_H0S + 1], n0[:])
                nc.vector.tensor_mul(t_d0[:], rz0[:, 1:2], t_d0[:])
                h0n = wrk.tile([128, 1], f32, tag="h0n", name="h0n")
                nc.vector.tensor_add(h0n[:], n0[:], t_d0[:])
                nc.sync.dma_start(out_h0n_d[:, :], h0n[:])
                h0n_bf = wrk.tile([128, 1], bf16, tag="h0n_bf", name="h0n_bf")
                nc.vector.tensor_copy(h0n_bf[:], h0n[:])

                # ---- GRU layer 1 gi1 partial ----
                p_p1 = ps_col.tile([128, 24], f32, tag="colA", name="p_p1")
                for m in range(24):
                    nc.tensor.matmul(p_p1[:, m:m + 1],
                                     wih1[:, m * 128:(m + 1) * 128],
                                     h0n_bf[:], start=True, stop=True)
                p1s = wrk.tile([128, 24], f32, tag="p1s", name="p1s")
                nc.vector.tensor_copy(p1s[:], p_p1[:])

                # ---- AllReduce B: gi1 partial ----
                ccB_in = dram.tile([128, 24], f32, tag="ccB_in", name="ccB_in")
                ccB_out = dram.tile([128, 24], f32, tag="ccB_out", name="ccB_out")
                nc.sync.dma_start(ccB_in[:], p1s[:])
                if parts == "nocc":
                    nc.sync.dma_start(ccB_out[:], ccB_in[:])
                else:
                    nc.gpsimd.collective_compute(
                        "AllReduce", mybir.AluOpType.add, replica_groups=RG,
                        ins=[ccB_in[:].opt()], outs=[ccB_out[:].opt()])
                gi1r = wrk.tile([128, 24], f32, tag="gi1r", name="gi1r")
                nc.sync.dma_start(gi1r[:], ccB_out[:])

                # ---- GRU layer 1 gates (full, replicated) ----
                gi1 = wrk.tile([128, 24], f32, tag="gi1", name="gi1")
                nc.vector.tensor_add(gi1[:], gi1r[:], small[:, C_BIH1:C_BIH1 + 24])
                gh1 = wrk.tile([128, 24], f32, tag="gh1", name="gh1")
                nc.vector.tensor_add(gh1[:], gh1r[:], small[:, C_BHH1:C_BHH1 + 24])
                t_rz1 = wrk.tile([128, 16], f32, tag="t_rz1", name="t_rz1")
                nc.vector.tensor_add(t_rz1[:], gi1[:, 0:16], gh1[:, 0:16])
                rz1 = wrk.tile([128, 16], f32, tag="rz1", name="rz1")
                nc.scalar.activation(rz1[:], t_rz1[:], AF.Sigmoid)
                t_n1 = wrk.tile([128, HT], f32, tag="t_n1", name="t_n1")
                nc.vector.tensor_mul(t_n1[:], rz1[:, 0:HT], gh1[:, 16:24])
                nc.vector.tensor_add(t_n1[:], t_n1[:], gi1[:, 16:24])
                n1 = wrk.tile([128, HT], f32, tag="n1", name="n1")
                nc.scalar.activation(n1[:], t_n1[:], AF.Tanh)
                t_d1 = wrk.tile([128, HT], f32, tag="t_d1", name="t_d1")
                nc.vector.tensor_sub(t_d1[:], small[:, C_QV:C_QV + 8], n1[:])
                nc.vector.tensor_mul(t_d1[:], rz1[:, HT:16], t_d1[:])
                h1n = wrk.tile([128, HT], f32, tag="h1n", name="h1n")
                nc.vector.tensor_add(h1n[:], n1[:], t_d1[:])
                nc.sync.dma_start(out_h1n_d[:, :], h1n[:])
                h1n_bf = wrk.tile([128, HT], bf16, tag="h1n_bf", name="h1n_bf")
                nc.vector.tensor_copy(h1n_bf[:], h1n[:])

                if parts == "gru":
                    continue

                # ---- dense (vocab-group order matches the Wd DMA stream) ----
                p_lg = ps_col.tile([128, VT], f32, tag="colA", name="p_lg")
                for m in range(VT):
                    for k in range(HT):
                        nc.tensor.matmul(
                            p_lg[:, m:m + 1],
                            wd[:, k * VC + m * 128:k * VC + (m + 1) * 128],
                            h1n_bf[:, k:k + 1],
                            start=(k == 0), stop=(k == HT - 1))
                logits = wrk.tile([128, VT], f32, tag="logits", name="logits")
                nc.vector.tensor_add(logits[:], p_lg[:], small[:, C_BD:C_BD + VT])

                # ---- softmax denominator ----
                el = wrk.tile([128, VT], f32, tag="el", name="el")
                nc.scalar.activation(el[:], logits[:], AF.Exp)
                zrow = wrk.tile([128, 1], f32, tag="zrow", name="zrow")
                nc.vector.tensor_reduce(zrow[:], el[:], mybir.AxisListType.X,
                                        mybir.AluOpType.add)
                zrow_bf = wrk.tile([128, 1], bf16, tag="zrow_bf", name="zrow_bf")
                nc.vector.tensor_copy(zrow_bf[:], zrow[:])
                p_zc = ps_tiny.tile([1, 1], f32, tag="tiny", name="p_zc")
                nc.tensor.matmul(p_zc[:], ones_col[:], zrow_bf[:],
                                 start=True, stop=True)
                zc = wrk.tile([1, 16], f32, tag="zc", name="zc")
                nc.vector.memset(zc[:], 0.0)
                nc.vector.tensor_copy(zc[0:1, 0:1], p_zc[:])

                # ---- AllReduce C ----
                ccC_in = dram.tile([1, 16], f32, tag="ccC_in", name="ccC_in")
                ccC_out = dram.tile([1, 16], f32, tag="ccC_out", name="ccC_out")
                nc.sync.dma_start(ccC_in[:], zc[:])
                if parts == "nocc":
                    nc.sync.dma_start(ccC_out[:], ccC_in[:])
                else:
                    nc.gpsimd.collective_compute(
                        "AllReduce", mybir.AluOpType.add, replica_groups=RG,
                        ins=[ccC_in[:].opt()], outs=[ccC_out[:].opt()])
                zt = wrk.tile([1, 1], f32, tag="zt", name="zt")
                nc.sync.dma_start(zt[:], ccC_out[0:1, 0:1])

                # ---- logp = logits - log(Z) ----
                lnz = wrk.tile([1, 1], f32, tag="lnz", name="lnz")
                nc.scalar.activation(lnz[:], zt[:], AF.Ln)
                p_nlz = ps_tiny.tile([128, 1], f32, tag="tiny", name="p_nlz")
                nc.tensor.matmul(p_nlz[:], negones_f[:], lnz[:],
                                 start=True, stop=True)
                nlz = wrk.tile([128, 1], f32, tag="nlz", name="nlz")
                nc.vector.tensor_copy(nlz[:], p_nlz[:])
                logp = wrk.tile([128, VT], f32, tag="logp", name="logp")
                nc.scalar.add(logp[:], logits[:], nlz[:])
                nc.sync.dma_start(out_logp_d[:, :], logp[:])

    nc.compile()
    return nc


def make_in_maps(input, hidden_state, enc_outputs, emb, Wq, Wk, wv,
                 W_ih0, W_hh0, b_ih0, b_hh0, W_ih1, W_hh1, b_ih1, b_hh1,
                 Wd, bd):
    input = np.asarray(input)
    hidden_state = np.asarray(hidden_state, np.float32)
    enc = np.asarray(enc_outputs, np.float32)
    emb = np.asarray(emb)
    idx = int(np.asarray(input).reshape(-1)[0])
    emb_row = np.asarray(emb[idx], np.float32)
    q = hidden_state[1, 0]
    h0 = hidden_state[0, 0]

    wk_sb = _to_sb(np.asarray(Wk, np.float32).astype(BF16), HT, H)
    wq_sb = _to_sb(np.asarray(Wq, np.float32).astype(BF16), HT, H)

    W_ih0T = np.asarray(W_ih0, np.float32).T    # [2048, 3072]
    W_hh0T = np.asarray(W_hh0, np.float32).T    # [1024, 3072]
    W_ih1T = np.asarray(W_ih1, np.float32).T    # [1024, 3072]
    W_hh1T = np.asarray(W_hh1, np.float32).T
    b_ih0 = np.asarray(b_ih0, np.float32)
    b_hh0 = np.asarray(b_hh0, np.float32)
    b_ih1 = np.asarray(b_ih1, np.float32)
    b_hh1 = np.asarray(b_hh1, np.float32)

    Wd = np.asarray(Wd, np.float32)
    bd = np.asarray(bd, np.float32)
    Wd_pad = np.zeros((H, VPAD), BF16)
    Wd_pad[:, :V] = Wd.astype(BF16)
    bd_pad = np.full((VPAD,), -30.0, np.float32)
    bd_pad[:V] = bd

    in_maps = []
    for c in range(NCORES):
        rows = np.r_[c * 128:(c + 1) * 128,
                     H + c * 128:H + (c + 1) * 128,
                     2 * H + c * 128:2 * H + (c + 1) * 128]
        small = np.zeros((128, NSMALL), np.float32)
        small[:, C_QV:C_QV + 8] = q.reshape(HT, 128).T
        small[:, C_H0V:C_H0V + 8] = h0.reshape(HT, 128).T
        small[:, C_EMBV:C_EMBV + 8] = emb_row.reshape(HT, 128).T
        small[:, C_H0S] = h0[c * 128:(c + 1) * 128]
        small[:, C_H1S] = q[c * 128:(c + 1) * 128]
        small[:, C_WV:C_WV + 8] = np.asarray(wv, np.float32).reshape(HT, 128).T
        small[:, C_BIH0:C_BIH0 + 3] = b_ih0[rows].reshape(3, 128).T
        small[:, C_BHH0:C_BHH0 + 3] = b_hh0[rows].reshape(3, 128).T
        small[:, C_BIH1:C_BIH1 + 24] = b_ih1.reshape(24, 128).T
        small[:, C_BHH1:C_BHH1 + 24] = b_hh1.reshape(24, 128).T
        small[:, C_BD:C_BD + VT] = bd_pad[c * VC:(c + 1) * VC].reshape(VT, 128).T

        enc_c = enc[c * LC:(c + 1) * LC]                     # [512, 1024]
        encT_sb = _to_sb(np.ascontiguousarray(enc_c.T).astype(BF16), HT, LC)
        encL_sb = _to_sb(enc_c.astype(BF16), LT, H)
        wih0_sb = _to_sb(np.ascontiguousarray(W_ih0T[:, rows]).astype(BF16),
                         16, 384)
        whh0_sb = _to_sb(np.ascontiguousarray(W_hh0T[:, rows]).astype(BF16),
                         8, 384)
        wih1_sb = np.ascontiguousarray(W_ih1T[c * 128:(c + 1) * 128]).astype(BF16)
        whh1_sb = np.ascontiguousarray(W_hh1T[c * 128:(c + 1) * 128]).astype(BF16)
        wd_sb = _to_sb(np.ascontiguousarray(Wd_pad[:, c * VC:(c + 1) * VC]),
                       HT, VC)
        in_maps.append({
            "small": small, "encT": encT_sb, "encL": encL_sb,
            "wk": wk_sb, "wq": wq_sb,
            "wih0": wih0_sb, "whh0": whh0_sb, "wih1": wih1_sb, "whh1": whh1_sb,
            "wd": wd_sb,
        })
    return in_maps


def assemble_outputs(results):
    logp = np.concatenate(
        [np.asarray(r["out_logp"]).T.reshape(VC) for r in results])[:V][None, :]
    h0n = np.concatenate([np.asarray(r["out_h0n"])[:, 0] for r in results])
    h1n = np.asarray(results[0]["out_h1n"]).T.reshape(H)
    new_hidden = np.stack([h0n, h1n])[:, None, :].astype(np.float32)
    return logp.astype(np.float32), new_hidden


def kernel(**inputs):
    from concourse import bass_utils
    if "nc" not in _CACHE:
        _CACHE["nc"] = build_nc()
    nc = _CACHE["nc"]
    in_maps = make_in_maps(**inputs)
    res = bass_utils.run_bass_kernel_spmd(
        nc, in_maps, core_ids=list(range(NCORES)))
    return assemble_outputs(res.results)


# revision 7
# speedup vs baseline: 1.2750x; 1.2750x over previous
"""Trainium2 Bass kernel for a single-step attention GRU decoder cell.

Math:
  emb_vec = relu(emb[idx]);  q = hidden[-1]
  feat = tanh(q@Wq + enc@Wk); scores = feat@wv; attn = softmax(scores, axis=L)
  context = attn^T @ enc
  x = [context; emb_vec];  h0n = GRU0(x, hidden[0]);  h1n = GRU1(h0n, hidden[1])
  logp = log_softmax(h1n @ Wd + bd)

Sharding over 8 NeuronCores:
  - attention: enc L-sharded (512 rows/core); AllReduce A combines the partial
    unnormalized context [1024], the partial softmax denominator Z, and the
    (input-column-sharded) gh1 partial for GRU layer 1.
  - GRU layer0: output rows sharded (128 rows of h0n per core, no collective).
  - GRU layer1: input(h0n)-column sharded; AllReduce B combines the gi1
    partial -> every core computes the full h1n.
  - dense: vocab sharded (6400 padded cols/core); AllReduce C combines the
    softmax denominator (a single scalar).
Weights are cast to bf16 on the host (matmul inputs); accumulation, softmax
and gate math stay f32. DMAs are ordered by critical path: packed small
inputs, enc/Wk chunks (feat), Wq, encL, GRU weights, then Wd chunks consumed
k-outer by the dense GEMV.
"""

import numpy as np
import ml_dtypes

NCORES = 8
H = 1024
HT = H // 128          # 8 h-tiles
L = 4096
LC = L // NCORES       # 512 rows per core
LT = LC // 128         # 4 l-tiles per core
V = 50257
VC = 6400              # padded vocab cols per core
VT = VC // 128         # 50 v-tiles per core
VPAD = VC * NCORES     # 51200

# packed small-input column layout (f32 [128, NSMALL])
C_QV, C_H0V, C_EMBV = 0, 8, 16
C_H0S, C_H1S = 24, 25
C_WV = 26
C_BIH0, C_BHH0 = 34, 37
C_BIH1, C_BHH1 = 40, 64
C_BD = 88
NSMALL = 138

BF16 = ml_dtypes.bfloat16

_CACHE = {}


def _to_sb(a, kt, f):
    """[kt*128, f] -> SBUF layout [128, kt*f] (k-tile-major along free dim)."""
    return np.ascontiguousarray(
        a.reshape(kt, 128, f).transpose(1, 0, 2).reshape(128, kt * f)
    )


def build_nc(reps=1, parts="full"):
    import concourse.bacc as bacc
    import concourse.mybir as mybir
    import concourse.tile as tile

    f32 = mybir.dt.float32
    bf16 = mybir.dt.bfloat16
    f8 = mybir.dt.float8e4
    AF = mybir.ActivationFunctionType

    nc = bacc.Bacc("TRN2", target_bir_lowering=False, debug=False,
                   num_devices=NCORES)

    def din(name, shape, dt=bf16):
        return nc.dram_tensor(name, shape, dt, kind="ExternalInput")

    small_d = din("small", [128, NSMALL], f32)
    encT_d = din("encT", [128, HT * LC])          # enc_c.T   (feat rhs)
    encL_d = din("encL", [128, LT * H])           # enc_c     (context lhsT)
    wk_d = din("wk", [128, HT * H])
    wq_d = din("wq", [128, HT * H])
    wih0_d = din("wih0", [128, 16 * 384])
    whh0_d = din("whh0", [128, 8 * 384])
    wih1_d = din("wih1", [128, 3072])
    whh1_d = din("whh1", [128, 3072])
    wd_d = din("wd", [128, HT * VC], f8)

    out_logp_d = nc.dram_tensor("out_logp", [128, VT], f32, kind="ExternalOutput")
    out_h0n_d = nc.dram_tensor("out_h0n", [128, 1], f32, kind="ExternalOutput")
    out_h1n_d = nc.dram_tensor("out_h1n", [128, HT], f32, kind="ExternalOutput")

    RG = [list(range(NCORES))]

    with tile.TileContext(nc) as tc:
        with (
            tc.tile_pool(name="inp", bufs=1) as inp,
            tc.tile_pool(name="wrk", bufs=1) as wrk,
            tc.tile_pool(name="ps_feat", bufs=2, space="PSUM") as ps_feat,
            tc.tile_pool(name="ps_col", bufs=2, space="PSUM") as ps_col,
            tc.tile_pool(name="ps_tiny", bufs=2, space="PSUM") as ps_tiny,
            tc.tile_pool(name="dram", bufs=1, space="DRAM") as dram,
        ):
            for _rep in range(reps):
                def sb(name, shape, dt):
                    return inp.tile(shape, dt, tag=name, name=name)

                # ---- DMAs, critical-path order ----
                small = sb("small", [128, NSMALL], f32)
                nc.sync.dma_start(small[:], small_d[:, :])
                encT = sb("encT", [128, HT * LC], bf16)
                wk = sb("wk", [128, HT * H], bf16)
                for k in range(HT):
                    nc.sync.dma_start(encT[:, k * LC:(k + 1) * LC],
                                      encT_d[:, k * LC:(k + 1) * LC])
                    nc.sync.dma_start(wk[:, k * H:(k + 1) * H],
                                      wk_d[:, k * H:(k + 1) * H])
                wq = sb("wq", [128, HT * H], bf16)
                for k in range(0, HT, 4):
                    nc.sync.dma_start(wq[:, k * H:(k + 4) * H],
                                      wq_d[:, k * H:(k + 4) * H])
                encL = sb("encL", [128, LT * H], bf16)
                nc.sync.dma_start(encL[:], encL_d[:, :])
                whh1 = sb("whh1", [128, 3072], bf16)
                nc.sync.dma_start(whh1[:], whh1_d[:, :])
                wih0 = sb("wih0", [128, 16 * 384], bf16)
                nc.sync.dma_start(wih0[:], wih0_d[:, :])
                whh0 = sb("whh0", [128, 8 * 384], bf16)
                nc.sync.dma_start(whh0[:], whh0_d[:, :])
                wih1 = sb("wih1", [128, 3072], bf16)
                nc.sync.dma_start(wih1[:], wih1_d[:, :])
                # Wd in 5 vocab-groups x 8 k-chunks so the dense GEMV can
                # finish each vocab-group as soon as its slice lands.
                wd = sb("wd", [128, HT * VC], f8)
                MG = VC // 5                     # 1280 cols per vocab-group
                for mg in range(5):
                    for k in range(HT):
                        nc.sync.dma_start(
                            wd[:, k * VC + mg * MG:k * VC + (mg + 1) * MG],
                            wd_d[:, k * VC + mg * MG:k * VC + (mg + 1) * MG])
                if parts == "dma":
                    continue

                # ---- constants ----
                ones_row = wrk.tile([1, 128], bf16, tag="ones_row", name="ones_row")
                nc.vector.memset(ones_row[:], 1.0)
                ones_col = wrk.tile([128, 1], bf16, tag="ones_col", name="ones_col")
                nc.vector.memset(ones_col[:], 1.0)
                negones_f = wrk.tile([1, 128], f32, tag="negones_f", name="negones_f")
                nc.vector.memset(negones_f[:], -1.0)

                # ---- casts ----
                q_bf = wrk.tile([128, HT], bf16, tag="q_bf", name="q_bf")
                nc.vector.tensor_copy(q_bf[:], small[:, C_QV:C_QV + 8])
                h0_bf = wrk.tile([128, HT], bf16, tag="h0_bf", name="h0_bf")
                nc.vector.tensor_copy(h0_bf[:], small[:, C_H0V:C_H0V + 8])
                emb_bf = wrk.tile([128, HT], bf16, tag="emb_bf", name="emb_bf")
                nc.scalar.activation(emb_bf[:], small[:, C_EMBV:C_EMBV + 8], AF.Relu)
                h1s_bf = wrk.tile([128, 1], bf16, tag="h1s_bf", name="h1s_bf")
                nc.vector.tensor_copy(h1s_bf[:], small[:, C_H1S:C_H1S + 1])
                wv_bf = wrk.tile([128, HT], bf16, tag="wv_bf", name="wv_bf")
                nc.vector.tensor_copy(wv_bf[:], small[:, C_WV:C_WV + 8])

                # ---- qWq:  qwq[h] = sum_k q[k] Wq[k, h]  -> [128, 8] ----
                p_qwq = ps_col.tile([128, HT], f32, tag="colA", name="p_qwq")
                for m in range(HT):
                    for k in range(HT):
                        nc.tensor.matmul(
                            p_qwq[:, m:m + 1],
                            wq[:, k * H + m * 128:k * H + (m + 1) * 128],
                            q_bf[:, k:k + 1],
                            start=(k == 0), stop=(k == HT - 1))
                qwq = wrk.tile([128, HT], f32, tag="qwq", name="qwq")
                nc.vector.tensor_copy(qwq[:], p_qwq[:])

                # ---- feat = tanh(qWq + enc@Wk), layout [h-part, l-free] ----
                feat = wrk.tile([128, HT * LC], bf16, tag="feat", name="feat")
                for m in range(HT):
                    p_feat = ps_feat.tile([128, LC], f32, tag="feat",
                                          name=f"p_feat{m}")
                    for k in range(HT):
                        nc.tensor.matmul(
                            p_feat[:],
                            wk[:, k * H + m * 128:k * H + (m + 1) * 128],
                            encT[:, k * LC:(k + 1) * LC],
                            start=(k == 0), stop=(k == HT - 1))
                    nc.scalar.activation(feat[:, m * LC:(m + 1) * LC], p_feat[:],
                                         AF.Tanh, bias=qwq[:, m:m + 1])

                # ---- scores (l-partition): s = feat^T @ wv -> [128, 4] ----
                p_sc = ps_col.tile([128, LT], f32, tag="colA", name="p_sc")
                for lt in range(LT):
                    for k in range(HT):
                        nc.tensor.matmul(
                            p_sc[:, lt:lt + 1],
                            feat[:, k * LC + lt * 128:k * LC + (lt + 1) * 128],
                            wv_bf[:, k:k + 1],
                            start=(k == 0), stop=(k == HT - 1))
                esc = wrk.tile([128, LT], bf16, tag="esc", name="esc")
                nc.scalar.activation(esc[:], p_sc[:], AF.Exp)

                # ---- partial context + partial Z -> [128, 9] (col 8 = Z) ----
                p_ctx = ps_col.tile([128, HT + 1], f32, tag="colA", name="p_ctx")
                for m in range(HT):
                    for lt in range(LT):
                        nc.tensor.matmul(
                            p_ctx[:, m:m + 1],
                            encL[:, lt * H + m * 128:lt * H + (m + 1) * 128],
                            esc[:, lt:lt + 1],
                            start=(lt == 0), stop=(lt == LT - 1))
                for lt in range(LT):
                    nc.tensor.matmul(
                        p_ctx[0:1, HT:HT + 1],
                        ones_col[:],
                        esc[:, lt:lt + 1],
                        start=(lt == 0), stop=(lt == LT - 1))
                ctxz = wrk.tile([128, HT + 1], f32, tag="ctxz", name="ctxz")
                nc.vector.memset(ctxz[:], 0.0)
                nc.vector.tensor_copy(ctxz[:, 0:HT], p_ctx[:, 0:HT])
                nc.vector.tensor_copy(ctxz[0:1, HT:HT + 1], p_ctx[0:1, HT:HT + 1])

                # ---- gh1 partial (depends only on whh1 + h1 slice) ----
                p_p2 = ps_col.tile([128, 24], f32, tag="colA", name="p_p2")
                for m in range(24):
                    nc.tensor.matmul(p_p2[:, m:m + 1],
                                     whh1[:, m * 128:(m + 1) * 128],
                                     h1s_bf[:], start=True, stop=True)
                p2s = wrk.tile([128, 24], f32, tag="p2s", name="p2s")
                nc.vector.tensor_copy(p2s[:], p_p2[:])

                # ---- AllReduce A: [ctx(8) | Z(1) | gh1_partial(24)] ----
                ccA_in = dram.tile([128, 33], f32, tag="ccA_in", name="ccA_in")
                ccA_out = dram.tile([128, 33], f32, tag="ccA_out", name="ccA_out")
                nc.sync.dma_start(ccA_in[:, 0:9], ctxz[:])
                nc.sync.dma_start(ccA_in[:, 9:33], p2s[:])
                if parts == "nocc":
                    nc.sync.dma_start(ccA_out[:], ccA_in[:])
                else:
                    nc.gpsimd.collective_compute(
                        "AllReduce", mybir.AluOpType.add, replica_groups=RG,
                        ins=[ccA_in[:].opt()], outs=[ccA_out[:].opt()])
                ctxz_r = wrk.tile([128, 9], f32, tag="ctxz_r", name="ctxz_r")
                nc.sync.dma_start(ctxz_r[:], ccA_out[:, 0:9])
                gh1r = wrk.tile([128, 24], f32, tag="gh1r", name="gh1r")
                nc.sync.dma_start(gh1r[:], ccA_out[:, 9:33])

                # ---- context = ctx_sum / Z ----
                zrec = wrk.tile([1, 1], f32, tag="zrec", name="zrec")
                nc.vector.reciprocal(zrec[:], ctxz_r[0:1, HT:HT + 1])
                zrec_bf = wrk.tile([1, 1], bf16, tag="zrec_bf", name="zrec_bf")
                nc.vector.tensor_copy(zrec_bf[:], zrec[:])
                p_zb = ps_tiny.tile([128, 1], f32, tag="tiny", name="p_zb")
                nc.tensor.matmul(p_zb[:], ones_row[:], zrec_bf[:],
                                 start=True, stop=True)
                zinv = wrk.tile([128, 1], f32, tag="zinv", name="zinv")
                nc.vector.tensor_copy(zinv[:], p_zb[:])
                ctx_bf = wrk.tile([128, HT], bf16, tag="ctx_bf", name="ctx_bf")
                nc.vector.tensor_scalar_mul(ctx_bf[:], ctxz_r[:, 0:HT], zinv[:])

                if parts == "attn":
                    continue

                # ---- GRU layer 0 (rows sharded) ----
                p_gi0 = ps_col.tile([128, 3], f32, tag="colA", name="p_gi0")
                for g in range(3):
                    for k in range(16):
                        rhs = (ctx_bf[:, k:k + 1] if k < 8
                               else emb_bf[:, k - 8:k - 7])
                        nc.tensor.matmul(
                            p_gi0[:, g:g + 1],
                            wih0[:, k * 384 + g * 128:k * 384 + (g + 1) * 128],
                            rhs, start=(k == 0), stop=(k == 15))
                p_gh0 = ps_col.tile([128, 3], f32, tag="colA", name="p_gh0")
                for g in range(3):
                    for k in range(8):
                        nc.tensor.matmul(
                            p_gh0[:, g:g + 1],
                            whh0[:, k * 384 + g * 128:k * 384 + (g + 1) * 128],
                            h0_bf[:, k:k + 1], start=(k == 0), stop=(k == 7))
                gi0 = wrk.tile([128, 3], f32, tag="gi0", name="gi0")
                nc.vector.tensor_add(gi0[:], p_gi0[:], small[:, C_BIH0:C_BIH0 + 3])
                gh0 = wrk.tile([128, 3], f32, tag="gh0", name="gh0")
                nc.vector.tensor_add(gh0[:], p_gh0[:], small[:, C_BHH0:C_BHH0 + 3])

                t_rz0 = wrk.tile([128, 2], f32, tag="t_rz0", name="t_rz0")
                nc.vector.tensor_add(t_rz0[:], gi0[:, 0:2], gh0[:, 0:2])
                rz0 = wrk.tile([128, 2], f32, tag="rz0", name="rz0")
                nc.scalar.activation(rz0[:], t_rz0[:], AF.Sigmoid)
                t_n0 = wrk.tile([128, 1], f32, tag="t_n0", name="t_n0")
                nc.vector.tensor_mul(t_n0[:], rz0[:, 0:1], gh0[:, 2:3])
                nc.vector.tensor_add(t_n0[:], t_n0[:], gi0[:, 2:3])
                n0 = wrk.tile([128, 1], f32, tag="n0", name="n0")
                nc.scalar.activation(n0[:], t_n0[:], AF.Tanh)
                # h0n = n0 + z*(h_prev - n0)
                t_d0 = wrk.tile([128, 1], f32, tag="t_d0", name="t_d0")
                nc.vector.tensor_sub(t_d0[:], small[:, C_H0S:C_H0S + 1], n0[:])
                nc.vector.tensor_mul(t_d0[:], rz0[:, 1:2], t_d0[:])
                h0n = wrk.tile([128, 1], f32, tag="h0n", name="h0n")
                nc.vector.tensor_add(h0n[:], n0[:], t_d0[:])
                nc.sync.dma_start(out_h0n_d[:, :], h0n[:])
                h0n_bf = wrk.tile([128, 1], bf16, tag="h0n_bf", name="h0n_bf")
                nc.vector.tensor_copy(h0n_bf[:], h0n[:])

                # ---- GRU layer 1 gi1 partial ----
                p_p1 = ps_col.tile([128, 24], f32, tag="colA", name="p_p1")
                for m in range(24):
                    nc.tensor.matmul(p_p1[:, m:m + 1],
                                     wih1[:, m * 128:(m + 1) * 128],
                                     h0n_bf[:], start=True, stop=True)
                p1s = wrk.tile([128, 24], f32, tag="p1s", name="p1s")
                nc.vector.tensor_copy(p1s[:], p_p1[:])

                # ---- AllReduce B: gi1 partial ----
                ccB_in = dram.tile([128, 24], f32, tag="ccB_in", name="ccB_in")
                ccB_out = dram.tile([128, 24], f32, tag="ccB_out", name="ccB_out")
                nc.sync.dma_start(ccB_in[:], p1s[:])
                if parts == "nocc":
                    nc.sync.dma_start(ccB_out[:], ccB_in[:])
                else:
                    nc.gpsimd.collective_compute(
                        "AllReduce", mybir.AluOpType.add, replica_groups=RG,
                        ins=[ccB_in[:].opt()], outs=[ccB_out[:].opt()])
                gi1r = wrk.tile([128, 24], f32, tag="gi1r", name="gi1r")
                nc.sync.dma_start(gi1r[:], ccB_out[:])

                # ---- GRU layer 1 gates (full, replicated) ----
                gi1 = wrk.tile([128, 24], f32, tag="gi1", name="gi1")
                nc.vector.tensor_add(gi1[:], gi1r[:], small[:, C_BIH1:C_BIH1 + 24])
                gh1 = wrk.tile([128, 24], f32, tag="gh1", name="gh1")
                nc.vector.tensor_add(gh1[:], gh1r[:], small[:, C_BHH1:C_BHH1 + 24])
                t_rz1 = wrk.tile([128, 16], f32, tag="t_rz1", name="t_rz1")
                nc.vector.tensor_add(t_rz1[:], gi1[:, 0:16], gh1[:, 0:16])
                rz1 = wrk.tile([128, 16], f32, tag="rz1", name="rz1")
                nc.scalar.activation(rz1[:], t_rz1[:], AF.Sigmoid)
                t_n1 = wrk.tile([128, HT], f32, tag="t_n1", name="t_n1")
                nc.vector.tensor_mul(t_n1[:], rz1[:, 0:HT], gh1[:, 16:24])
                nc.vector.tensor_add(t_n1[:], t_n1[:], gi1[:, 16:24])
                n1 = wrk.tile([128, HT], f32, tag="n1", name="n1")
                nc.scalar.activation(n1[:], t_n1[:], AF.Tanh)
                t_d1 = wrk.tile([128, HT], f32, tag="t_d1", name="t_d1")
                nc.vector.tensor_sub(t_d1[:], small[:, C_QV:C_QV + 8], n1[:])
                nc.vector.tensor_mul(t_d1[:], rz1[:, HT:16], t_d1[:])
                h1n = wrk.tile([128, HT], f32, tag="h1n", name="h1n")
                nc.vector.tensor_add(h1n[:], n1[:], t_d1[:])
                nc.sync.dma_start(out_h1n_d[:, :], h1n[:])
                h1n_f8 = wrk.tile([128, HT], f8, tag="h1n_f8", name="h1n_f8")
                nc.vector.tensor_copy(h1n_f8[:], h1n[:])

                if parts == "gru":
                    continue

                # ---- dense (vocab-group order matches the Wd DMA stream) ----
                p_lg = ps_col.tile([128, VT], f32, tag="colA", name="p_lg")
                for m in range(VT):
                    for k in range(HT):
                        nc.tensor.matmul(
                            p_lg[:, m:m + 1],
                            wd[:, k * VC + m * 128:k * VC + (m + 1) * 128],
                            h1n_f8[:, k:k + 1],
                            start=(k == 0), stop=(k == HT - 1))
                logits = wrk.tile([128, VT], f32, tag="logits", name="logits")
                nc.vector.scalar_tensor_tensor(
                    logits[:], p_lg[:], 1.0 / 256.0, small[:, C_BD:C_BD + VT],
                    mybir.AluOpType.mult, mybir.AluOpType.add)

                # ---- softmax denominator ----
                el = wrk.tile([128, VT], f32, tag="el", name="el")
                nc.scalar.activation(el[:], logits[:], AF.Exp)
                zrow = wrk.tile([128, 1], f32, tag="zrow", name="zrow")
                nc.vector.tensor_reduce(zrow[:], el[:], mybir.AxisListType.X,
                                        mybir.AluOpType.add)
                zrow_bf = wrk.tile([128, 1], bf16, tag="zrow_bf", name="zrow_bf")
                nc.vector.tensor_copy(zrow_bf[:], zrow[:])
                p_zc = ps_tiny.tile([1, 1], f32, tag="tiny", name="p_zc")
                nc.tensor.matmul(p_zc[:], ones_col[:], zrow_bf[:],
                                 start=True, stop=True)
                zc = wrk.tile([1, 16], f32, tag="zc", name="zc")
                nc.vector.memset(zc[:], 0.0)
                nc.vector.tensor_copy(zc[0:1, 0:1], p_zc[:])

                # ---- AllReduce C ----
                ccC_in = dram.tile([1, 16], f32, tag="ccC_in", name="ccC_in")
                ccC_out = dram.tile([1, 16], f32, tag="ccC_out", name="ccC_out")
                nc.sync.dma_start(ccC_in[:], zc[:])
                if parts == "nocc":
                    nc.sync.dma_start(ccC_out[:], ccC_in[:])
                else:
                    nc.gpsimd.collective_compute(
                        "AllReduce", mybir.AluOpType.add, replica_groups=RG,
                        ins=[ccC_in[:].opt()], outs=[ccC_out[:].opt()])
                zt = wrk.tile([1, 1], f32, tag="zt", name="zt")
                nc.sync.dma_start(zt[:], ccC_out[0:1, 0:1])

                # ---- logp = logits - log(Z) ----
                lnz = wrk.tile([1, 1], f32, tag="lnz", name="lnz")
                nc.scalar.activation(lnz[:], zt[:], AF.Ln)
                p_nlz = ps_tiny.tile([128, 1], f32, tag="tiny", name="p_nlz")
                nc.tensor.matmul(p_nlz[:], negones_f[:], lnz[:],
                                 start=True, stop=True)
                nlz = wrk.tile([128, 1], f32, tag="nlz", name="nlz")
                nc.vector.tensor_copy(nlz[:], p_nlz[:])
                logp = wrk.tile([128, VT], f32, tag="logp", name="logp")
                nc.scalar.add(logp[:], logits[:], nlz[:])
                nc.sync.dma_start(out_logp_d[:, :], logp[:])

    nc.compile()
    return nc


def make_in_maps(input, hidden_state, enc_outputs, emb, Wq, Wk, wv,
                 W_ih0, W_hh0, b_ih0, b_hh0, W_ih1, W_hh1, b_ih1, b_hh1,
                 Wd, bd):
    input = np.asarray(input)
    hidden_state = np.asarray(hidden_state, np.float32)
    enc = np.asarray(enc_outputs, np.float32)
    emb = np.asarray(emb)
    idx = int(np.asarray(input).reshape(-1)[0])
    emb_row = np.asarray(emb[idx], np.float32)
    q = hidden_state[1, 0]
    h0 = hidden_state[0, 0]

    wk_sb = _to_sb(np.asarray(Wk, np.float32).astype(BF16), HT, H)
    wq_sb = _to_sb(np.asarray(Wq, np.float32).astype(BF16), HT, H)

    W_ih0T = np.asarray(W_ih0, np.float32).T    # [2048, 3072]
    W_hh0T = np.asarray(W_hh0, np.float32).T    # [1024, 3072]
    W_ih1T = np.asarray(W_ih1, np.float32).T    # [1024, 3072]
    W_hh1T = np.asarray(W_hh1, np.float32).T
    b_ih0 = np.asarray(b_ih0, np.float32)
    b_hh0 = np.asarray(b_hh0, np.float32)
    b_ih1 = np.asarray(b_ih1, np.float32)
    b_hh1 = np.asarray(b_hh1, np.float32)

    Wd = np.asarray(Wd, np.float32)
    bd = np.asarray(bd, np.float32)
    F8 = ml_dtypes.float8_e4m3fn
    Wd_pad = np.zeros((H, VPAD), F8)
    Wd_pad[:, :V] = (Wd * 256.0).astype(F8)
    bd_pad = np.full((VPAD,), -30.0, np.float32)
    bd_pad[:V] = bd

    in_maps = []
    for c in range(NCORES):
        rows = np.r_[c * 128:(c + 1) * 128,
                     H + c * 128:H + (c + 1) * 128,
                     2 * H + c * 128:2 * H + (c + 1) * 128]
        small = np.zeros((128, NSMALL), np.float32)
        small[:, C_QV:C_QV + 8] = q.reshape(HT, 128).T
        small[:, C_H0V:C_H0V + 8] = h0.reshape(HT, 128).T
        small[:, C_EMBV:C_EMBV + 8] = emb_row.reshape(HT, 128).T
        small[:, C_H0S] = h0[c * 128:(c + 1) * 128]
        small[:, C_H1S] = q[c * 128:(c + 1) * 128]
        small[:, C_WV:C_WV + 8] = np.asarray(wv, np.float32).reshape(HT, 128).T
        small[:, C_BIH0:C_BIH0 + 3] = b_ih0[rows].reshape(3, 128).T
        small[:, C_BHH0:C_BHH0 + 3] = b_hh0[rows].reshape(3, 128).T
        small[:, C_BIH1:C_BIH1 + 24] = b_ih1.reshape(24, 128).T
        small[:, C_BHH1:C_BHH1 + 24] = b_hh1.reshape(24, 128).T
        small[:, C_BD:C_BD + VT] = bd_pad[c * VC:(c + 1) * VC].reshape(VT, 128).T

        enc_c = enc[c * LC:(c + 1) * LC]                     # [512, 1024]
        encT_sb = _to_sb(np.ascontiguousarray(enc_c.T).astype(BF16), HT, LC)
        encL_sb = _to_sb(enc_c.astype(BF16), LT, H)
        wih0_sb = _to_sb(np.ascontiguousarray(W_ih0T[:, rows]).astype(BF16),
                         16, 384)
        whh0_sb = _to_sb(np.ascontiguousarray(W_hh0T[:, rows]).astype(BF16),
                         8, 384)
        wih1_sb = np.ascontiguousarray(W_ih1T[c * 128:(c + 1) * 128]).astype(BF16)
        whh1_sb = np.ascontiguousarray(W_hh1T[c * 128:(c + 1) * 128]).astype(BF16)
        wd_sb = _to_sb(np.ascontiguousarray(Wd_pad[:, c * VC:(c + 1) * VC]),
                       HT, VC)
        in_maps.append({
            "small": small, "encT": encT_sb, "encL": encL_sb,
            "wk": wk_sb, "wq": wq_sb,
            "wih0": wih0_sb, "whh0": whh0_sb, "wih1": wih1_sb, "whh1": whh1_sb,
            "wd": wd_sb,
        })
    return in_maps


def assemble_outputs(results):
    logp = np.concatenate(
        [np.asarray(r["out_logp"]).T.reshape(VC) for r in results])[:V][None, :]
    h0n = np.concatenate([np.asarray(r["out_h0n"])[:, 0] for r in results])
    h1n = np.asarray(results[0]["out_h1n"]).T.reshape(H)
    new_hidden = np.stack([h0n, h1n])[:, None, :].astype(np.float32)
    return logp.astype(np.float32), new_hidden


def kernel(**inputs):
    from concourse import bass_utils
    if "nc" not in _CACHE:
        _CACHE["nc"] = build_nc()
    nc = _CACHE["nc"]
    in_maps = make_in_maps(**inputs)
    res = bass_utils.run_bass_kernel_spmd(
        nc, in_maps, core_ids=list(range(NCORES)))
    return assemble_outputs(res.results)
